# revision 35
# baseline (speedup 1.0000x reference)
"""RNN-T Joiner kernel for 8 Trainium2 NeuronCores.

out[b,t,u,:] = tanh(enc[b,t,:] + pred[b,u,:]) @ W.T + b

Sharding: 2 cores per batch, each takes half the t range (200 t), so every
core owns one batch and 20000 (t,u) cells. Data path is bf16
(enc/pred/W/logits/output) with f32 PSUM accumulation; the 2e-2 rel-err
budget dwarfs the ~0.5% bf16 error.

Per-core engine budget (PE matmuls ~140us are the wall):
  - producers (logit = enc[c,t] + pred[c,u], broadcast add): DVE broadcast
    APs run at 1x (~3.5us/32t block), so one add per block goes to GpSimd
    (~9us incl drain) and four late-block adds run fused on ACT (per-t tanh
    with per-partition enc bias); the rest stay on DVE.
  - ACT: big in-place tanh per (ck, block), ~2.9us
  - PE: psum[cells, v] += logit[c, cells].T @ W[c, v]; 4 chunk matmuls per
    <=128-cell tile, 4 tiles (banks) per psum group, double-buffered groups
  - DVE: one bias-add per group (psum f32 + bias f32 -> bf16 sbuf), ~2.3us
  - DMA: ~512KB bf16 stores per group

The t-blocks open with an 8t micro-block so the PE starts ~8us in instead
of ~30us; the consts DMA is split so enc/pred land before W/bias.
"""

import sys

sys.path.insert(0, "/opt/trn_rl_repo")

import numpy as np
import ml_dtypes

import concourse.bass as bass
import concourse.bacc as bacc
import concourse.mybir as mybir
from concourse.tile import TileContext
from concourse.bass_utils import run_bass_kernel_spmd

B, T, U, C, V = 4, 400, 100, 512, 512
NCORES = 8
TSC = T // 2  # 200 t per core (2 cores per batch)
P = 128
CK = C // P  # 4 contraction chunks
CELLS = TSC * U  # 20000 cells per core
F32 = mybir.dt.float32
BF16 = mybir.dt.bfloat16
BF = ml_dtypes.bfloat16

BLOCK_T = [8, 24, 32, 32, 32, 32, 32, 8]
BLOCK_CELLS = [bt * U for bt in BLOCK_T]
BLOCK_C0 = np.cumsum([0] + BLOCK_CELLS).tolist()
NBLK = len(BLOCK_T)
MAXBC = max(BLOCK_CELLS)

# mm tiles: (blk, local_offset, m, global_cell_start); <=128 cells, within
# one block so each tile reads one logit SBUF tile
TILES = []
GROUPS = []  # groups never span blocks: PE deps stay within one block
for _blk in range(NBLK):
    _c = BLOCK_CELLS[_blk]
    _bt = []
    for _s in range(0, _c, P):
        _bt.append((_blk, _s, min(P, _c - _s), BLOCK_C0[_blk] + _s))
    TILES.extend(_bt)
    # never leave a trailing 1-tile group: its ~0.9us of matmuls can't
    # cover the previous group's ~2.4us consumer+psum-release, stalling
    # the PE at every block boundary. 25 -> 4,4,4,4,4,3,2.
    _i = 0
    while _i < len(_bt):
        _rem = len(_bt) - _i
        _take = 4 if (_rem > 5 or _rem == 4) else (3 if _rem == 5 else _rem)
        GROUPS.append(_bt[_i : _i + _take])
        _i += _take

# packed consts layout (bf16 columns); enc/pred first so a small leading
# DMA unblocks producers before the W/bias bulk lands. enc is stored
# DUPLICATED pairwise ([ck, t, 2]) so the broadcast add's innermost dim is
# a step-1 run of 2 bf16 on all three operands -> DVE 2x_1P packing mode
# (halves the add cost vs a plain [ck, t] layout whose u-broadcast is
# step-0 and falls back to 1x).
T0B = BLOCK_T[0]  # first micro-block's t count
E0_OFF = 0  # enc2 for block 0, t-major [t, ck, 2] -> 8*4*2
PRED_OFF = E0_OFF + T0B * CK * 2  # [ck, u] -> 4*100 (viewed [ck, 50, 2])
W_OFF = PRED_OFF + CK * U  # [ck, v] -> 4*512
E1_OFF = W_OFF + CK * V  # enc2 for t>=8, [t, ck, 2] -> 192*4*2
BIAS_OFF = E1_OFF + (TSC - T0B) * CK * 2  # f32 bias [4,512] as 2x bf16 cols
BIASB_OFF = BIAS_OFF + 2 * 4 * V  # bf16 bias replicated [4, 512]
NCOL = BIASB_OFF + 4 * V  # 10192

# DVE runs ~21.7us of work per 32t block vs the PE's 21.6us — a 100%
# lockstep that drips stalls into the PE. Divert one consumer group per
# 32t-ish block to ACT (psum -> bf16 copy) + a cheap all-bf16 2x DVE add.
ACT_GROUPS = {4, 9, 16, 23, 30, 37}

# input DMA split: [enc2(block0)+pred] -> [W] -> [enc2 rest] -> [biases].
# The first ~120KB unblocks block 0's producers at ~1us and W lands before
# the first matmul needs it.
DMA_SPLITS = [W_OFF, E1_OFF, BIAS_OFF, NCOL]

# producer unit engines (unit = blk*4 + ck): with 2x-mode adds (~1.8us)
# DVE runs everything. GpSimd is a net loss: it shares DVE's SBUF port and
# each of its ops stretched a concurrent DVE op to 2-3x. Fused-ACT
# producers (12us per block) are also a net loss.
GPS_UNITS = set()
# block 1's ck3 runs fused on the (early-idle) ACT engine, shortening the
# serial DVE producer chain while the pipeline fills
ACTF_UNITS = {7}

_cache = {}


def _build():
    nc = bacc.Bacc("TRN2", target_bir_lowering=False, debug=False)
    consts = nc.declare_dram_parameter("consts", [P, NCOL], BF16, isOutput=False)
    out = nc.declare_dram_parameter("out", [TSC, U, V], BF16, isOutput=True)
    ob = out.ap().rearrange("t u v -> (t u) v")  # [20000, 512]

    with TileContext(nc) as tc:
        with (
            tc.tile_pool(name="consts", bufs=1) as cpool,
            tc.tile_pool(name="logit", bufs=3) as logit_pool,
            tc.tile_pool(name="logit_s", bufs=2) as logit_s_pool,
            tc.tile_pool(name="osb", bufs=4) as out_pool,
            tc.tile_pool(name="csp", bufs=2) as copy_pool,
            tc.tile_pool(name="warm", bufs=1) as warm_pool,
            tc.tile_pool(name="psum", bufs=2, space="PSUM") as psum_pool,
        ):
            # 8 dummy matmuls on garbage SBUF warm the PE clock gate (HAM)
            # during the input-DMA/producer ramp, so the first real matmuls
            # run at 2.4GHz instead of 1.2
            wt = warm_pool.tile([P, 640], BF16, tag="wt")
            wo = warm_pool.tile([P, 8], F32, tag="wo")
            nc.gpsimd.memset(wt, 0.0)
            ps_d = psum_pool.tile([P, 4 * V], F32, tag="ps")
            for _ in range(8):
                nc.tensor.matmul(
                    ps_d[:, :V], lhsT=wt[:, :P], rhs=wt[:, P : P + V],
                    start=True, stop=True,
                )
            nc.vector.tensor_copy(out=wo[0:1, :], in_=ps_d[0:1, :8])

            cs = cpool.tile([P, NCOL], BF16, tag="cs")
            prev = 0
            for b in DMA_SPLITS:
                nc.sync.dma_start(
                    out=cs[:, prev:b], in_=consts.ap()[:, prev:b]
                )
                prev = b

            ev0 = cs[:, E0_OFF : E0_OFF + T0B * CK * 2].rearrange(
                "p (t ck two) -> p t ck two", ck=CK, two=2
            )
            ev1 = cs[:, E1_OFF : E1_OFF + (TSC - T0B) * CK * 2].rearrange(
                "p (t ck two) -> p t ck two", ck=CK, two=2
            )
            pview = cs[:, PRED_OFF : PRED_OFF + CK * U].rearrange(
                "p (ck u) -> p ck u", ck=CK
            )
            pview2 = cs[:, PRED_OFF : PRED_OFF + CK * U].rearrange(
                "p (ck h two) -> p ck h two", ck=CK, two=2
            )

            def enc_slice(ck, t0, bt):  # [P, bt, 2] for t range [t0, t0+bt)
                if t0 < T0B:
                    return ev0[:, t0 : t0 + bt, ck, :]
                return ev1[:, t0 - T0B : t0 - T0B + bt, ck, :]
            wview = cs[:, W_OFF : W_OFF + CK * V].rearrange(
                "p (ck v) -> p ck v", ck=CK
            )
            bias_f32 = cs[:, BIAS_OFF : BIAS_OFF + 2 * 4 * V].bitcast(F32)
            bias_bf = cs[:, BIASB_OFF : BIASB_OFF + 4 * V]

            lg = {}  # (blk, ck) -> [P, MAXBC] bf16 tile

            def emit_producer(blk, ck):
                bt = BLOCK_T[blk]
                t0 = sum(BLOCK_T[:blk])
                ncell = bt * U
                if bt <= 8:  # micro-blocks get their own slots
                    lgt = logit_s_pool.tile([P, 8 * U], BF16, tag=f"lgs{ck}")
                else:
                    lgt = logit_pool.tile([P, MAXBC], BF16, tag=f"lg{ck}")
                lg[(blk, ck)] = lgt
                unit = blk * 4 + ck
                if unit in ACTF_UNITS:
                    v3 = lgt[:, :ncell].rearrange("p (t u) -> p t u", t=bt)
                    esl = enc_slice(ck, t0, bt)
                    for t in range(bt):
                        nc.scalar.activation(
                            out=v3[:, t, :],
                            in_=pview[:, ck, :],
                            func=mybir.ActivationFunctionType.Tanh,
                            bias=esl[:, t, 0:1],
                        )
                else:
                    # pairwise-duplicated enc keeps every innermost AP a
                    # step-1 run of 2 -> DVE 2x_1P
                    v4d = lgt[:, :ncell].rearrange(
                        "p (t h two) -> p t h two", t=bt, two=2
                    )
                    e_col = (
                        enc_slice(ck, t0, bt)
                        .unsqueeze(2)
                        .broadcast_to([P, bt, U // 2, 2])
                    )
                    p_row = (
                        pview2[:, ck]
                        .unsqueeze(1)
                        .broadcast_to([P, bt, U // 2, 2])
                    )
                    eng = nc.gpsimd if unit in GPS_UNITS else nc.vector
                    eng.tensor_add(out=v4d, in0=e_col, in1=p_row)
                    nc.scalar.activation(
                        out=lgt[:, :ncell],
                        in_=lgt[:, :ncell],
                        func=mybir.ActivationFunctionType.Tanh,
                    )

            def emit_group(g):
                tiles = GROUPS[g]
                ps = psum_pool.tile([P, 4 * V], F32, tag="ps")
                for j, (blk, off, m, _) in enumerate(tiles):
                    for ck in range(CK):
                        nc.tensor.matmul(
                            ps[:m, j * V : (j + 1) * V],
                            lhsT=lg[(blk, ck)][:, off : off + m],
                            rhs=wview[:, ck, :],
                            start=(ck == 0),
                            stop=(ck == CK - 1),
                        )
                ncol = len(tiles) * V
                osb = out_pool.tile([P, 4 * V], BF16, tag="osb")
                if g in ACT_GROUPS:
                    tmp = copy_pool.tile([P, 4 * V], BF16, tag="csp")
                    nc.scalar.copy(out=tmp[:, :ncol], in_=ps[:, :ncol])
                    nc.vector.tensor_add(
                        out=osb[:, :ncol],
                        in0=tmp[:, :ncol],
                        in1=bias_bf[:, :ncol],
                    )
                else:
                    nc.vector.tensor_add(
                        out=osb[:, :ncol],
                        in0=ps[:, :ncol],
                        in1=bias_f32[:, :ncol],
                    )
                # one DMA per run of full tiles; ragged tiles DMA alone
                j = 0
                while j < len(tiles):
                    if tiles[j][2] == P:
                        j1 = j
                        while j1 < len(tiles) and tiles[j1][2] == P:
                            j1 += 1
                        c0 = tiles[j][3]
                        n = j1 - j
                        dst = ob[c0 : c0 + n * P, :].rearrange(
                            "(k p) v -> p k v", p=P
                        )
                        src = osb[:, j * V : j1 * V].rearrange(
                            "p (k v) -> p k v", v=V
                        )
                        nc.sync.dma_start(out=dst, in_=src)
                        j = j1
                    else:
                        blk, off, m, c0 = tiles[j]
                        nc.sync.dma_start(
                            out=ob[c0 : c0 + m, :],
                            in_=osb[:m, j * V : (j + 1) * V],
                        )
                        j += 1

            # emit groups as soon as the blocks they read are emitted; the
            # cheap final micro-block is produced early (after block 2) so
            # the PE never waits on producers at the very end
            next_g = 0
            for blk in range(NBLK - 1):
                for ck in range(CK):
                    emit_producer(blk, ck)
                if blk == 2:
                    for ck in range(CK):
                        emit_producer(NBLK - 1, ck)
                while next_g < len(GROUPS) and GROUPS[next_g][-1][0] <= blk:
                    emit_group(next_g)
                    next_g += 1
            while next_g < len(GROUPS):
                emit_group(next_g)
                next_g += 1
    nc.compile()
    return nc


def _install_ntff_hook():
    """This image's antenv lacks axon_hooks; wire the ctypes NTFF hook from
    trn_boot against the axon PJRT .so so trace=True works."""
    if "antenv.axon_hooks" in sys.modules:
        return
    import types

    holder = [None]
    mod = types.ModuleType("antenv.axon_hooks")
    mod.set_axon_ntff_profile_hook = lambda h: holder.__setitem__(0, h)
    mod.get_axon_ntff_profile_hook = lambda: holder[0]
    sys.modules["antenv.axon_hooks"] = mod
    try:
        sys.path.insert(0, "/root/.axon_site/trn_agent_boot")
        from trn_boot import _ntff_profile_via_ctypes

        mod.set_axon_ntff_profile_hook(
            _ntff_profile_via_ctypes("/opt/axon/libaxon_pjrt.so")
        )
    except Exception as e:  # degrade to no tracing
        print(f"NTFF hook install failed: {e}", file=sys.stderr)


def _run(in_maps, trace=False, tmpdir=None):
    if "nc" not in _cache:
        _cache["nc"] = _build()
    if trace:
        _install_ntff_hook()
    return run_bass_kernel_spmd(
        _cache["nc"], in_maps, list(range(NCORES)), trace=trace, tmpdir=tmpdir
    )


def make_in_maps(encoder_out, predictor_out, W, b):
    encoder_out = np.asarray(encoder_out, dtype=np.float32)
    predictor_out = np.asarray(predictor_out, dtype=np.float32)
    W = np.asarray(W, dtype=np.float32)
    b = np.asarray(b, dtype=np.float32)

    # [p, ck, v] <- W[v, ck*P+p]
    w_pack = W.reshape(V, CK, P).transpose(2, 1, 0).reshape(P, CK * V)
    bias_rep = np.tile(b, (P, 4, 1)).reshape(P, 4 * V).astype(np.float32)
    bias_bf = bias_rep.view(BF)  # raw f32 bytes as 2x bf16 cols

    in_maps = []
    for i in range(NCORES):
        bb, half = i // 2, i % 2
        base = np.zeros((P, NCOL), BF)
        base[:, W_OFF : W_OFF + CK * V] = w_pack.astype(BF)
        base[:, BIAS_OFF : BIAS_OFF + 2 * 4 * V] = bias_bf
        base[:, BIASB_OFF : BIASB_OFF + 4 * V] = bias_rep.astype(BF)
        enc_s = encoder_out[bb, half * TSC : (half + 1) * TSC, :]  # [t, c]
        enc_p = enc_s.reshape(TSC, CK, P).transpose(2, 0, 1)  # [p, t, ck]
        enc_d = np.repeat(enc_p[..., None], 2, axis=-1)  # [p, t, ck, 2]
        base[:, E0_OFF : E0_OFF + T0B * CK * 2] = (
            enc_d[:, :T0B].reshape(P, -1).astype(BF)
        )
        base[:, E1_OFF : E1_OFF + (TSC - T0B) * CK * 2] = (
            enc_d[:, T0B:].reshape(P, -1).astype(BF)
        )
        base[:, PRED_OFF : PRED_OFF + CK * U] = (
            predictor_out[bb].reshape(U, CK, P).transpose(2, 1, 0).reshape(P, -1)
        ).astype(BF)
        in_maps.append({"consts": base})
    return in_maps


def gather(results):
    full = np.empty((B, T, U, V), np.float32)
    for i in range(NCORES):
        bb, half = i // 2, i % 2
        full[bb, half * TSC : (half + 1) * TSC] = np.asarray(
            results[i]["out"]
        ).astype(np.float32)
    return full


def kernel(encoder_out, predictor_out, W, b):
    in_maps = make_in_maps(encoder_out, predictor_out, W, b)
    res = _run(in_maps, trace=False)
    return gather(res.results)
